# revision 57
# baseline (speedup 1.0000x reference)
"""Causal multi-head attention (B=2, S=2048, D=1024, H=16) on 8 trn2 cores.

Sharding: core = (batch b = core//4, head-group g = core%4 of 4 heads).
Per core: Q/K/V projections for its 4 heads (Wq/Wk/Wv column-sharded),
causal attention, and the output projection against the row-shard of Wo.
The 4 per-batch partials are summed on the host (the TP all-reduce).

All activations/weights are bf16 (host-converted): halves DMA traffic and
runs every matmul at the full 1-cycle/row PE rate regardless of free size.

Cost-model-driven layout (matmul cost = out free size only; weights and
contraction depth are free):
  - Q^T/K^T projections land as (features, tokens) tiles (lhsT = weights,
    rhs = activation chunk, free = 512 tokens).
  - scores S^T (tk partitions, tq free) per head pair via row-tiled 64-
    partition matmuls; exp via one ACT op per (pair, tile) into bf16 p2;
    causal diagonal masked on GPSIMD.
  - PV is flipped vs the classic layout: out O = (tq tokens partitions,
    dk+1 free) with lhsT = P^T tile (exp'd scores, already in the right
    layout) and rhs = V (tokens, dk) + ones column.  Free size is 65
    instead of 512, halving the PE cost of PV.  The ones column makes
    O[:, 64] the softmax denominator: normalization is a per-partition
    reciprocal + tensor_scalar multiply on DVE - no DMA round trips.
  - O (tokens, E) is transposed back to (E, tokens) with cheap PE
    transposes (128 cycles each) for the output projection.
Emission is software-pipelined per 512-token chunk so DMA, PE, ACT, DVE
and GPSIMD overlap: scores are zipped with fill work (this chunk's V
projection, the previous chunk's deferred transpose/outproj, the next
chunk's Q/K projection), the first 7 score tiles of the next chunk are
pre-scored during this chunk's diagonal phase (double-buffered p2 tags)
so the exp stream never starves, and the last chunk runs pair-0 scores,
then pair-1 scores zipped with pair-0 PV, then pair-1 PV with the
transpose/outproj cascade.  All DMAs ride the SP queue.
"""

import numpy as np

B, S, D, H = 2, 2048, 1024, 16
DK = D // H               # 64
N_CORES = 8
G = 4                     # head-groups (cores per batch)
HPG = H // G              # 4 heads per core
NPAIR = HPG // 2          # 2 head-pairs per core
E = HPG * DK              # 256 per-core projection width
TQ = 512                  # tq chunk (PSUM bank width in f32)
NQ = S // TQ              # 4 tq chunks
TK = 128                  # tk tile
NK = S // TK              # 16 tk tiles
NKD = D // 128            # 8 contraction tiles over D

_NC_CACHE = None
MM_TRACE = []          # label per emitted matmul, in program order
_CUR = [""]


def _build():
    import concourse.tile as tile
    from concourse import bacc, mybir

    F32 = mybir.dt.float32
    BF16 = mybir.dt.bfloat16
    EXP = mybir.ActivationFunctionType.Exp

    nc = bacc.Bacc("TRN2", debug=False, num_devices=N_CORES)

    MM_TRACE.clear()
    _orig_mm = nc.tensor.matmul

    def _mm(*a, **k):
        MM_TRACE.append(_CUR[0])
        return _orig_mm(*a, **k)

    nc.tensor.matmul = _mm

    def _lbl(s):
        _CUR[0] = s

    xqT = nc.dram_tensor("xqT", (D, S), BF16, kind="ExternalInput").ap()
    xkT = nc.dram_tensor("xkT", (D, S), BF16, kind="ExternalInput").ap()
    xvT = nc.dram_tensor("xvT", (D, S), BF16, kind="ExternalInput").ap()
    wql = nc.dram_tensor("wql", (2, 128, NKD, 128), BF16,
                         kind="ExternalInput").ap()
    wkl = nc.dram_tensor("wkl", (2, 128, NKD, 128), BF16,
                         kind="ExternalInput").ap()
    wvr = nc.dram_tensor("wvr", (D, E), BF16, kind="ExternalInput").ap()
    wor = nc.dram_tensor("wor", (E, D), BF16, kind="ExternalInput").ap()
    mask4 = nc.dram_tensor("mask4", (4, TK, TQ), BF16, kind="ExternalInput").ap()
    ident = nc.dram_tensor("ident", (128, 128), BF16, kind="ExternalInput").ap()
    out = nc.dram_tensor("out", (S, D), BF16, kind="ExternalOutput").ap()

    with tile.TileContext(nc) as tc:
        with tc.tile_pool(name="consts", bufs=1) as consts, \
             tc.tile_pool(name="stage", bufs=2) as stage, \
             tc.tile_pool(name="p2p", bufs=1) as p2p, \
             tc.tile_pool(name="onrm", bufs=2) as onrm_pool, \
             tc.tile_pool(name="norm", bufs=4) as norm, \
             tc.tile_pool(name="otp", bufs=4) as otp, \
             tc.tile_pool(name="osb", bufs=2) as osb_pool, \
             tc.tile_pool(name="psum", bufs=1, space="PSUM") as psum:

            wql_sb = consts.tile([128, 2, NKD, 128], BF16)
            wkl_sb = consts.tile([128, 2, NKD, 128], BF16)
            wvr_sb = consts.tile([128, NKD, E], BF16)
            wor_sb = consts.tile([128, 2, D], BF16)
            mask_sb = consts.tile([128, 4, TQ], BF16)
            ident_sb = consts.tile([128, 128], BF16)
            scr = consts.tile([1, 2], BF16)
            scr2 = consts.tile([1, 2], F32)
            vaug = consts.tile([128, NK, HPG, DK + 1], BF16)
            kT_sb = [consts.tile([128, S], BF16, name=f"kT{j}") for j in range(NPAIR)]
            qTs = {}

            xq_r = xqT.rearrange("(k p) t -> p k t", p=128)
            xk_r = xkT.rearrange("(k p) t -> p k t", p=128)
            xv_r = xvT.rearrange("(k p) t -> p k t", p=128)

            stages = {}

            def emit_load(n, halves):
                t0, t1 = n * TQ, (n + 1) * TQ
                st = {}
                for nm, src in (("q", xq_r), ("k", xk_r), ("v", xv_r)):
                    t = stage.tile([128, NKD, TQ], BF16, name=f"x{nm}_{n}",
                                   tag=f"x{nm}")
                    if halves:
                        hk = NKD // 2
                        nc.sync.dma_start(t[:, 0:hk, :], src[:, 0:hk, t0:t1])
                        nc.sync.dma_start(t[:, hk:NKD, :], src[:, hk:NKD, t0:t1])
                    else:
                        nc.sync.dma_start(t[:], src[:, :, t0:t1])
                    st[nm] = t
                stages[n] = st

            def emit_proj_qk_group(n, nm, w_sb, dst, j, copy_eng):
                xs = stages[n][nm]
                _lbl(f"proj_{nm}{n}j{j}")
                pp = psum.tile([128, TQ], F32, name=f"pp{nm}_{n}_{j}",
                               tag="s", bufs=2)
                for k in range(NKD):
                    nc.tensor.matmul(
                        pp[:], w_sb[:, j, k, :],
                        xs[:, k, :],
                        start=(k == 0), stop=(k == NKD - 1),
                    )
                if nm == "q":
                    qt = consts.tile([128, TQ], BF16, name=f"qT_{n}_{j}",
                                     tag=f"qT{j}", bufs=2)
                    qTs[(n, j)] = qt
                    copy_eng.tensor_copy(qt[:], pp[:])
                else:
                    copy_eng.tensor_copy(dst[j][:, n * TQ:(n + 1) * TQ], pp[:])

            def emit_proj_v_group(n, dm):
                xs = stages[n]["v"]
                m = 4 * n + dm
                _lbl(f"projv{n}m{m}")
                vp = psum.tile([128, E], F32, name=f"vp_{m}", tag="s", bufs=2)
                for k in range(NKD):
                    nc.tensor.matmul(
                        vp[:], xs[:, k, dm * TK:(dm + 1) * TK],
                        wvr_sb[:, k, :],
                        start=(k == 0), stop=(k == NKD - 1),
                    )
                nc.vector.tensor_copy(
                    vaug[:, m, :, 0:DK],
                    vp.rearrange("p (h e) -> p h e", h=HPG))

            def qk_thunks(n):
                th = []
                for j in range(NPAIR):
                    th.append(lambda n=n, j=j: emit_proj_qk_group(
                        n, "q", wql_sb, None, j, nc.vector))
                for j in range(NPAIR):
                    th.append(lambda n=n, j=j: emit_proj_qk_group(
                        n, "k", wkl_sb, kT_sb, j, nc.vector))
                return th

            def projv_thunks(n):
                return [lambda n=n, dm=dm: emit_proj_v_group(n, dm)
                        for dm in range(4)]

            def emit_score_tile(n, j, i):
                o = i - 4 * n
                _lbl(f"sc{n}j{j}i{i}")
                f0 = max(0, o * TK)
                s2 = psum.tile([128, 2, TQ], F32, name=f"s2_{n}_{j}_{i}",
                               tag="s2", bufs=2)
                for hh in range(2):
                    nc.tensor.matmul(
                        s2[:, hh, f0:],
                        kT_sb[j][hh * 64:(hh + 1) * 64, i * TK:(i + 1) * TK],
                        qTs[(n, j)][hh * 64:(hh + 1) * 64, f0:TQ],
                        start=True, stop=True,
                    )
                p2 = p2p.tile([128, 2, TQ], BF16, name=f"p2_{j}_{i}_{n}",
                              tag=f"p2_{j}_{i}", bufs=2 if i < 7 else 1)
                nc.scalar.activation(p2[:, :, f0:], s2[:, :, f0:], EXP,
                                     scale=0.125)
                if o >= 0:
                    nc.vector.tensor_mul(
                        p2[:, 0, f0:], p2[:, 0, f0:], mask_sb[:, o, f0:])
                    nc.gpsimd.tensor_mul(
                        p2[:, 1, f0:], p2[:, 1, f0:], mask_sb[:, o, f0:])
                return p2

            def emit_pv_group(n, h, u, p2s, o_nrm):
                j, hh = divmod(h, 2)
                _lbl(f"pv{n}h{h}u{u}")
                t = 4 * n + u
                opv = psum.tile([128, DK + 1], F32, name=f"opv_{n}_{h}_{u}",
                                tag="opv", bufs=2)
                for i in range(t + 1):
                    nc.tensor.matmul(
                        opv[:],
                        p2s[i][:, hh, u * TK:(u + 1) * TK],
                        vaug[:, i, h, :],
                        start=(i == 0), stop=(i == t),
                    )
                rec = norm.tile([128, 1], F32, name=f"rec_{n}_{h}_{u}",
                                tag="rec")
                nc.vector.reciprocal_approx_fast(rec[:], opv[:, DK:DK + 1])
                nc.vector.tensor_scalar_mul(
                    o_nrm[:, u, h * DK:(h + 1) * DK], opv[:, 0:DK], rec[:])

            def emit_tr_u(n, u, o_nrm):
                m = 4 * n + u
                _lbl(f"tr{n}u{u}")
                oT = otp.tile([128, 2, TK], BF16, name=f"oT_{m}", tag="oT")
                for c in range(2):
                    tp = psum.tile([128, TK], BF16, name=f"tp_{m}_{c}",
                                   tag="s", bufs=2)
                    nc.tensor.transpose(
                        tp[:], o_nrm[:, u, c * TK:(c + 1) * TK], ident_sb[:])
                    nc.vector.tensor_copy(oT[:, c, :], tp[:])
                return oT

            def emit_op_u(n, u, oT, o_sb):
                m = 4 * n + u
                _lbl(f"op{n}u{u}")
                for c in range(2):
                    op = psum.tile([128, TQ], F32, name=f"op_{m}_{c}",
                                   tag="s", bufs=2)
                    for jj in range(2):
                        nc.tensor.matmul(
                            op[:], oT[:, jj, :],
                            wor_sb[:, jj, c * TQ:(c + 1) * TQ],
                            start=(jj == 0), stop=(jj == 1),
                        )
                    nc.vector.tensor_copy(o_sb[:, u, c * TQ:(c + 1) * TQ],
                                          op[:])

            def emit_out_dma(n, o_sb, uu):
                # half-chunk output store (tiles uu, uu+1) on the SP queue
                nc.sync.dma_start(
                    out[n * TQ + uu * TK:n * TQ + (uu + 2) * TK, :]
                    .rearrange("(u p) d -> p u d", p=128),
                    o_sb[:, uu:uu + 2, :])

            def emit_out_dma_u(n, o_sb, u):
                nc.sync.dma_start(
                    out[n * TQ + u * TK:n * TQ + (u + 1) * TK, :],
                    o_sb[:, u, :])

            # ---- prologue: weight + chunk-0/1 loads on SP, ACT exp-table
            # warmup, chunk-0 projections ----
            nc.sync.dma_start(wql_sb[:, 0], wql[0])
            nc.vector.memset(scr[:], 0.5)
            nc.vector.memset(vaug[:, :, :, DK:DK + 1], 1.0)
            nc.scalar.activation(scr2[:], scr[:], EXP, scale=0.125)
            xq0 = stage.tile([128, NKD, TQ], BF16, name="xq_0", tag="xq")
            xk0 = stage.tile([128, NKD, TQ], BF16, name="xk_0", tag="xk")
            xv0 = stage.tile([128, NKD, TQ], BF16, name="xv_0", tag="xv")
            for q in range(4):
                nc.sync.dma_start(xq0[:, 2 * q:2 * q + 2, :],
                                  xq_r[:, 2 * q:2 * q + 2, 0:TQ])
            nc.sync.dma_start(wkl_sb[:, 0], wkl[0])
            for q in range(4):
                nc.sync.dma_start(xk0[:, 2 * q:2 * q + 2, :],
                                  xk_r[:, 2 * q:2 * q + 2, 0:TQ])
            nc.sync.dma_start(wql_sb[:, 1], wql[1])
            nc.sync.dma_start(wkl_sb[:, 1], wkl[1])
            nc.sync.dma_start(wvr_sb[:], wvr.rearrange("(k p) e -> p k e", p=128))
            for hh2 in range(2):
                nc.sync.dma_start(xv0[:, 4 * hh2:4 * hh2 + 4, :],
                                  xv_r[:, 4 * hh2:4 * hh2 + 4, 0:TQ])
            nc.sync.dma_start(mask_sb[:], mask4.rearrange("o p f -> p o f"))
            stages[0] = {"q": xq0, "k": xk0, "v": xv0}
            emit_load(1, halves=False)
            nc.sync.dma_start(ident_sb[:], ident)
            nc.sync.dma_start(wor_sb[:], wor.rearrange("(j p) f -> p j f", p=128))

            emit_proj_qk_group(0, "q", wql_sb, None, 0, nc.vector)
            emit_proj_qk_group(0, "k", wkl_sb, kT_sb, 0, nc.vector)
            emit_proj_qk_group(0, "q", wql_sb, None, 1, nc.vector)
            emit_proj_qk_group(0, "k", wkl_sb, kT_sb, 1, nc.vector)

            # ---- main loop ----
            # Scores are zipped with fill work (this chunk's V projection,
            # the previous chunk's deferred transpose/outproj, the next
            # chunk's Q/K projection).  PV groups are emitted at their
            # earliest-ready point: group u right after diagonal score tile
            # 4n+u.  transpose/outproj of chunk n are deferred into chunk
            # n+1's fill (except the last chunk, which trails one step
            # behind its PV groups).
            deferred = []
            pre = {0: [], 1: []}
            from collections import deque
            for n in range(NQ):
                if n + 2 < NQ:
                    emit_load(n + 2, halves=False)
                o_nrm = onrm_pool.tile([128, 4, E], BF16, name=f"onrm_{n}",
                                       tag="on")
                o_sb = osb_pool.tile([128, 4, D], BF16, name=f"osb_{n}",
                                     tag="osb")
                fill = deque(projv_thunks(n))
                if n + 1 < NQ:
                    fill.extend(qk_thunks(n + 1))
                fill.extend(deferred)
                deferred = []
                n_tiles = 4 * n + 4
                p2s = pre
                pre = {0: [], 1: []}
                # pre-score requests for chunk n+1 (first 4 tiles) once this
                # chunk's fill is exhausted (guarantees q/k of n+1 emitted)
                pre_req = deque(range(7)) if n + 1 < NQ else deque()
                i0 = len(p2s[0])
                if n == NQ - 1:
                    # last chunk: all pair-0 scores first, then pair-1 scores
                    # zipped with pair-0 PV, then pair-1 PV with the
                    # transpose/outproj cascade.  Pair-0 exps complete while
                    # the pair-1 scores stream, so PV never outruns ACT.
                    nA = n_tiles - i0
                    for idx, i in enumerate(range(i0, n_tiles)):
                        p2s[0].append(emit_score_tile(n, 0, i))
                        k = -(-len(fill) // (2 * nA - idx))
                        for _ in range(k):
                            if fill:
                                fill.popleft()()
                    for i in range(i0, n_tiles):
                        p2s[1].append(emit_score_tile(n, 1, i))
                        k = -(-len(fill) // (n_tiles - i))
                        for _ in range(k):
                            if fill:
                                fill.popleft()()
                        if i >= 4 * n + 1:
                            u = i - 4 * n - 1
                            emit_pv_group(n, 0, u, p2s[0], o_nrm)
                            emit_pv_group(n, 1, u, p2s[0], o_nrm)
                    emit_pv_group(n, 0, 3, p2s[0], o_nrm)
                    emit_pv_group(n, 1, 3, p2s[0], o_nrm)
                    for u in range(3):
                        emit_pv_group(n, 2, u, p2s[1], o_nrm)
                        emit_pv_group(n, 3, u, p2s[1], o_nrm)
                        oT = emit_tr_u(n, u, o_nrm)
                        emit_op_u(n, u, oT, o_sb)
                        emit_out_dma_u(n, o_sb, u)
                    m = 4 * n + 3
                    _lbl(f"tr{n}u3")
                    oT = otp.tile([128, 2, TK], BF16, name=f"oT_{m}", tag="oT")
                    tp = psum.tile([128, TK], BF16, name=f"tp_{m}_0",
                                   tag="s2", bufs=2)
                    nc.tensor.transpose(tp[:], o_nrm[:, 3, 0:TK], ident_sb[:])
                    nc.scalar.copy(oT[:, 0, :], tp[:])
                    emit_pv_group(n, 2, 3, p2s[1], o_nrm)
                    emit_pv_group(n, 3, 3, p2s[1], o_nrm)
                    _lbl(f"tr{n}u3")
                    tp = psum.tile([128, TK], BF16, name=f"tp_{m}_1",
                                   tag="s2", bufs=2)
                    nc.tensor.transpose(tp[:], o_nrm[:, 3, TK:2 * TK],
                                        ident_sb[:])
                    nc.scalar.copy(oT[:, 1, :], tp[:])
                    _lbl(f"op{n}u3")
                    for c in range(2):
                        op = psum.tile([128, TQ], F32, name=f"op_{m}_{c}",
                                       tag="s", bufs=2)
                        for jj in range(2):
                            nc.tensor.matmul(
                                op[:], oT[:, jj, :],
                                wor_sb[:, jj, c * TQ:(c + 1) * TQ],
                                start=(jj == 0), stop=(jj == 1),
                            )
                        nc.vector.tensor_copy(
                            o_sb[:, 3, c * TQ:(c + 1) * TQ], op[:])
                        nc.sync.dma_start(
                            out[n * TQ + 3 * TK:n * TQ + 4 * TK,
                                c * TQ:(c + 1) * TQ],
                            o_sb[:, 3, c * TQ:(c + 1) * TQ])
                    continue
                next_u = 0
                lag = 1 if n == 0 else 0
                for i in range(i0, n_tiles):
                    p2s[0].append(emit_score_tile(n, 0, i))
                    if i - lag >= 0:
                        p2s[1].append(emit_score_tile(n, 1, i - lag))
                    o = i - 4 * n
                    tiles_left = n_tiles - i
                    k = -(-len(fill) // tiles_left)
                    for _ in range(k):
                        if fill:
                            fill.popleft()()
                    while next_u <= i - 4 * n - 1 - lag:
                        u = next_u
                        for h in range(HPG):
                            emit_pv_group(n, h, u, p2s[h // 2], o_nrm)
                        next_u += 1
                    if o >= 0 and not fill and pre_req:
                        ii = pre_req.popleft()
                        pre[0].append(emit_score_tile(n + 1, 0, ii))
                        pre[1].append(emit_score_tile(n + 1, 1, ii))
                if True:
                    while fill:
                        fill.popleft()()
                    if lag:
                        p2s[1].append(emit_score_tile(n, 1, n_tiles - 1))
                    while pre_req:
                        ii = pre_req.popleft()
                        pre[0].append(emit_score_tile(n + 1, 0, ii))
                        pre[1].append(emit_score_tile(n + 1, 1, ii))
                    while next_u <= 3:
                        for h in range(HPG):
                            emit_pv_group(n, h, next_u, p2s[h // 2], o_nrm)
                        next_u += 1

                    def mk_units(n, o_nrm, o_sb):
                        units = []
                        st = {}
                        for u in range(4):
                            def tr(u=u):
                                st[u] = emit_tr_u(n, u, o_nrm)
                            def op(u=u):
                                emit_op_u(n, u, st[u], o_sb)
                                if u % 2 == 1:
                                    emit_out_dma(n, o_sb, u - 1)
                            units += [tr, op]
                        return units
                    deferred = mk_units(n, o_nrm, o_sb)

    nc.compile()
    return nc


def _get_nc():
    global _NC_CACHE
    if _NC_CACHE is None:
        _NC_CACHE = _build()
    return _NC_CACHE


def kernel(query, key, value, mask, Wq, Wk, Wv, Wo):
    import ml_dtypes
    from concourse.bass_utils import run_bass_kernel_spmd

    BF = ml_dtypes.bfloat16

    query = np.asarray(query, dtype=np.float32)
    key = np.asarray(key, dtype=np.float32)
    value = np.asarray(value, dtype=np.float32)
    mask = np.asarray(mask)
    Wq = np.asarray(Wq, dtype=np.float32)
    Wk = np.asarray(Wk, dtype=np.float32)
    Wv = np.asarray(Wv, dtype=np.float32)
    Wo = np.asarray(Wo, dtype=np.float32)

    # 4 diagonal-offset masks (tk-local partition p, tq-chunk col f):
    # keep iff tk_global <= tq_global  <=>  f >= o*128 + p.
    m4 = np.empty((4, TK, TQ), dtype=BF)
    msub = np.asarray(mask[0, :TQ, :TQ] != 0, dtype=np.float32)  # (tq, tk)
    for o in range(4):
        m4[o] = msub[:, o * TK:(o + 1) * TK].T.astype(BF)
    ident = np.eye(128, dtype=BF)

    xT = {}
    for b in range(B):
        xT[("q", b)] = np.ascontiguousarray(query[b].T.astype(BF))
        xT[("k", b)] = np.ascontiguousarray(key[b].T.astype(BF))
        xT[("v", b)] = np.ascontiguousarray(value[b].T.astype(BF))

    in_maps = []
    for core in range(N_CORES):
        b, g = divmod(core, G)
        sl = slice(g * E, (g + 1) * E)
        in_maps.append({
            "xqT": xT[("q", b)],
            "xkT": xT[("k", b)],
            "xvT": xT[("v", b)],
            "wql": np.ascontiguousarray(
                Wq[sl, :].T.astype(BF).reshape(NKD, 128, 2, 128)
                .transpose(2, 1, 0, 3)),
            "wkl": np.ascontiguousarray(
                Wk[sl, :].T.astype(BF).reshape(NKD, 128, 2, 128)
                .transpose(2, 1, 0, 3)),
            "wvr": np.ascontiguousarray(Wv[sl, :].T.astype(BF)),
            "wor": np.ascontiguousarray(Wo[:, sl].T.astype(BF)),
            "mask4": m4,
            "ident": ident,
        })

    nc = _get_nc()
    res = run_bass_kernel_spmd(nc, in_maps, core_ids=list(range(N_CORES)))

    out = np.zeros((B, S, D), dtype=np.float32)
    for core in range(N_CORES):
        out[core // G] += res.results[core]["out"].astype(np.float32)
    return out


# revision 63
# speedup vs baseline: 1.0045x; 1.0045x over previous
"""Causal multi-head attention (B=2, S=2048, D=1024, H=16) on 8 trn2 cores.

Sharding: core = (batch b = core//4, head-group g = core%4 of 4 heads).
Per core: Q/K/V projections for its 4 heads (Wq/Wk/Wv column-sharded),
causal attention, and the output projection against the row-shard of Wo.
The 4 per-batch partials are summed on the host (the TP all-reduce).

All activations/weights are bf16 (host-converted): halves DMA traffic and
runs every matmul at the full 1-cycle/row PE rate regardless of free size.

Cost-model-driven layout (matmul cost = out free size only; weights and
contraction depth are free):
  - Q^T/K^T projections land as (features, tokens) tiles (lhsT = weights,
    rhs = activation chunk, free = 512 tokens).
  - scores S^T (tk partitions, tq free) per head pair via row-tiled 64-
    partition matmuls; exp via one ACT op per (pair, tile) into bf16 p2;
    causal diagonal masked on GPSIMD.
  - PV is flipped vs the classic layout: out O = (tq tokens partitions,
    dk+1 free) with lhsT = P^T tile (exp'd scores, already in the right
    layout) and rhs = V (tokens, dk) + ones column.  Free size is 65
    instead of 512, halving the PE cost of PV.  The ones column makes
    O[:, 64] the softmax denominator: normalization is a per-partition
    reciprocal + tensor_scalar multiply on DVE - no DMA round trips.
  - O (tokens, E) is transposed back to (E, tokens) with cheap PE
    transposes (128 cycles each) for the output projection.
Emission is software-pipelined per 512-token chunk so DMA, PE, ACT, DVE
and GPSIMD overlap: scores are zipped with fill work (this chunk's V
projection, the previous chunk's deferred transpose/outproj, the next
chunk's Q/K projection), the first 7 score tiles of the next chunk are
pre-scored during this chunk's diagonal phase (double-buffered p2 tags)
so the exp stream never starves, and the last chunk runs pair-0 scores,
then pair-1 scores zipped with pair-0 PV, then pair-1 PV with the
transpose/outproj cascade.  All DMAs ride the SP queue.
"""

import numpy as np

B, S, D, H = 2, 2048, 1024, 16
DK = D // H               # 64
N_CORES = 8
G = 4                     # head-groups (cores per batch)
HPG = H // G              # 4 heads per core
NPAIR = HPG // 2          # 2 head-pairs per core
E = HPG * DK              # 256 per-core projection width
TQ = 512                  # tq chunk (PSUM bank width in f32)
NQ = S // TQ              # 4 tq chunks
TK = 128                  # tk tile
NK = S // TK              # 16 tk tiles
NKD = D // 128            # 8 contraction tiles over D

_NC_CACHE = None
MM_TRACE = []          # label per emitted matmul, in program order
_CUR = [""]


def _build():
    import concourse.tile as tile
    from concourse import bacc, mybir

    F32 = mybir.dt.float32
    BF16 = mybir.dt.bfloat16
    EXP = mybir.ActivationFunctionType.Exp

    nc = bacc.Bacc("TRN2", debug=False, num_devices=N_CORES)

    MM_TRACE.clear()
    _orig_mm = nc.tensor.matmul

    def _mm(*a, **k):
        MM_TRACE.append(_CUR[0])
        return _orig_mm(*a, **k)

    nc.tensor.matmul = _mm

    def _lbl(s):
        _CUR[0] = s

    xqT = nc.dram_tensor("xqT", (D, S), BF16, kind="ExternalInput").ap()
    xkT = nc.dram_tensor("xkT", (D, S), BF16, kind="ExternalInput").ap()
    xvT = nc.dram_tensor("xvT", (D, S), BF16, kind="ExternalInput").ap()
    wql = nc.dram_tensor("wql", (2, 128, NKD, 128), BF16,
                         kind="ExternalInput").ap()
    wkl = nc.dram_tensor("wkl", (2, 128, NKD, 128), BF16,
                         kind="ExternalInput").ap()
    wvr = nc.dram_tensor("wvr", (D, E), BF16, kind="ExternalInput").ap()
    wor = nc.dram_tensor("wor", (E, D), BF16, kind="ExternalInput").ap()
    ident = nc.dram_tensor("ident", (128, 128), BF16, kind="ExternalInput").ap()
    out = nc.dram_tensor("out", (S, D), BF16, kind="ExternalOutput").ap()

    with tile.TileContext(nc) as tc:
        with tc.tile_pool(name="consts", bufs=1) as consts, \
             tc.tile_pool(name="stage", bufs=2) as stage, \
             tc.tile_pool(name="p2p", bufs=1) as p2p, \
             tc.tile_pool(name="onrm", bufs=2) as onrm_pool, \
             tc.tile_pool(name="norm", bufs=4) as norm, \
             tc.tile_pool(name="otp", bufs=4) as otp, \
             tc.tile_pool(name="osb", bufs=2) as osb_pool, \
             tc.tile_pool(name="psum", bufs=1, space="PSUM") as psum:

            wql_sb = consts.tile([128, 2, NKD, 128], BF16)
            wkl_sb = consts.tile([128, 2, NKD, 128], BF16)
            wvr_sb = consts.tile([128, NKD, E], BF16)
            wor_sb = consts.tile([128, 2, D], BF16)
            mask_sb = consts.tile([128, 4, TQ], BF16)
            it_sb = consts.tile([128, TQ], mybir.dt.int16)
            ident_sb = consts.tile([128, 128], BF16)
            scr = consts.tile([1, 2], BF16)
            scr2 = consts.tile([1, 2], F32)
            vaug = consts.tile([128, NK, HPG, DK + 1], BF16)
            kT_sb = [consts.tile([128, S], BF16, name=f"kT{j}") for j in range(NPAIR)]
            qTs = {}

            xq_r = xqT.rearrange("(k p) t -> p k t", p=128)
            xk_r = xkT.rearrange("(k p) t -> p k t", p=128)
            xv_r = xvT.rearrange("(k p) t -> p k t", p=128)

            stages = {}

            def emit_load(n, halves):
                t0, t1 = n * TQ, (n + 1) * TQ
                st = {}
                for nm, src in (("q", xq_r), ("k", xk_r), ("v", xv_r)):
                    t = stage.tile([128, NKD, TQ], BF16, name=f"x{nm}_{n}",
                                   tag=f"x{nm}")
                    if halves:
                        hk = NKD // 2
                        nc.sync.dma_start(t[:, 0:hk, :], src[:, 0:hk, t0:t1])
                        nc.sync.dma_start(t[:, hk:NKD, :], src[:, hk:NKD, t0:t1])
                    else:
                        nc.sync.dma_start(t[:], src[:, :, t0:t1])
                    st[nm] = t
                stages[n] = st

            def emit_proj_qk_group(n, nm, w_sb, dst, j, copy_eng):
                xs = stages[n][nm]
                _lbl(f"proj_{nm}{n}j{j}")
                pp = psum.tile([128, TQ], F32, name=f"pp{nm}_{n}_{j}",
                               tag="s", bufs=2)
                for k in range(NKD):
                    nc.tensor.matmul(
                        pp[:], w_sb[:, j, k, :],
                        xs[:, k, :],
                        start=(k == 0), stop=(k == NKD - 1),
                    )
                if nm == "q":
                    qt = consts.tile([128, TQ], BF16, name=f"qT_{n}_{j}",
                                     tag=f"qT{j}", bufs=2)
                    qTs[(n, j)] = qt
                    copy_eng.tensor_copy(qt[:], pp[:])
                else:
                    copy_eng.tensor_copy(dst[j][:, n * TQ:(n + 1) * TQ], pp[:])

            def emit_proj_v_group(n, dm):
                xs = stages[n]["v"]
                m = 4 * n + dm
                _lbl(f"projv{n}m{m}")
                vp = psum.tile([128, E], F32, name=f"vp_{m}", tag="s", bufs=2)
                for k in range(NKD):
                    nc.tensor.matmul(
                        vp[:], xs[:, k, dm * TK:(dm + 1) * TK],
                        wvr_sb[:, k, :],
                        start=(k == 0), stop=(k == NKD - 1),
                    )
                nc.vector.tensor_copy(
                    vaug[:, m, :, 0:DK],
                    vp.rearrange("p (h e) -> p h e", h=HPG))

            def qk_thunks(n):
                th = []
                for j in range(NPAIR):
                    th.append(lambda n=n, j=j: emit_proj_qk_group(
                        n, "q", wql_sb, None, j, nc.vector))
                for j in range(NPAIR):
                    th.append(lambda n=n, j=j: emit_proj_qk_group(
                        n, "k", wkl_sb, kT_sb, j, nc.vector))
                return th

            def projv_thunks(n):
                return [lambda n=n, dm=dm: emit_proj_v_group(n, dm)
                        for dm in range(4)]

            def emit_score_tile(n, j, i):
                o = i - 4 * n
                _lbl(f"sc{n}j{j}i{i}")
                f0 = max(0, o * TK)
                s2 = psum.tile([128, 2, TQ], F32, name=f"s2_{n}_{j}_{i}",
                               tag="s2", bufs=2)
                for hh in range(2):
                    nc.tensor.matmul(
                        s2[:, hh, f0:],
                        kT_sb[j][hh * 64:(hh + 1) * 64, i * TK:(i + 1) * TK],
                        qTs[(n, j)][hh * 64:(hh + 1) * 64, f0:TQ],
                        start=True, stop=True,
                    )
                p2 = p2p.tile([128, 2, TQ], BF16, name=f"p2_{j}_{i}_{n}",
                              tag=f"p2_{j}_{i}", bufs=2 if i < 7 else 1)
                nc.scalar.activation(p2[:, :, f0:], s2[:, :, f0:], EXP,
                                     scale=0.125)
                if o >= 0:
                    nc.vector.tensor_mul(
                        p2[:, 0, f0:], p2[:, 0, f0:], mask_sb[:, o, f0:])
                    nc.gpsimd.tensor_mul(
                        p2[:, 1, f0:], p2[:, 1, f0:], mask_sb[:, o, f0:])
                return p2

            def emit_pv_group(n, h, u, p2s, o_nrm):
                j, hh = divmod(h, 2)
                _lbl(f"pv{n}h{h}u{u}")
                t = 4 * n + u
                opv = psum.tile([128, DK + 1], F32, name=f"opv_{n}_{h}_{u}",
                                tag="opv", bufs=2)
                for i in range(t + 1):
                    nc.tensor.matmul(
                        opv[:],
                        p2s[i][:, hh, u * TK:(u + 1) * TK],
                        vaug[:, i, h, :],
                        start=(i == 0), stop=(i == t),
                    )
                rec = norm.tile([128, 1], F32, name=f"rec_{n}_{h}_{u}",
                                tag="rec")
                nc.vector.reciprocal_approx_fast(rec[:], opv[:, DK:DK + 1])
                nc.vector.tensor_scalar_mul(
                    o_nrm[:, u, h * DK:(h + 1) * DK], opv[:, 0:DK], rec[:])

            def emit_tr_u(n, u, o_nrm):
                m = 4 * n + u
                _lbl(f"tr{n}u{u}")
                oT = otp.tile([128, 2, TK], BF16, name=f"oT_{m}", tag="oT")
                for c in range(2):
                    tp = psum.tile([128, TK], BF16, name=f"tp_{m}_{c}",
                                   tag="s", bufs=2)
                    nc.tensor.transpose(
                        tp[:], o_nrm[:, u, c * TK:(c + 1) * TK], ident_sb[:])
                    nc.vector.tensor_copy(oT[:, c, :], tp[:])
                return oT

            def emit_op_u(n, u, oT, o_sb):
                m = 4 * n + u
                _lbl(f"op{n}u{u}")
                for c in range(2):
                    op = psum.tile([128, TQ], F32, name=f"op_{m}_{c}",
                                   tag="s", bufs=2)
                    for jj in range(2):
                        nc.tensor.matmul(
                            op[:], oT[:, jj, :],
                            wor_sb[:, jj, c * TQ:(c + 1) * TQ],
                            start=(jj == 0), stop=(jj == 1),
                        )
                    nc.vector.tensor_copy(o_sb[:, u, c * TQ:(c + 1) * TQ],
                                          op[:])

            def emit_out_dma(n, o_sb, uu):
                # half-chunk output store (tiles uu, uu+1) on the SP queue
                nc.sync.dma_start(
                    out[n * TQ + uu * TK:n * TQ + (uu + 2) * TK, :]
                    .rearrange("(u p) d -> p u d", p=128),
                    o_sb[:, uu:uu + 2, :])

            def emit_out_dma_u(n, o_sb, u):
                nc.sync.dma_start(
                    out[n * TQ + u * TK:n * TQ + (u + 1) * TK, :],
                    o_sb[:, u, :])

            # ---- prologue: weight + chunk-0/1 loads on SP, ACT exp-table
            # warmup, chunk-0 projections ----
            nc.sync.dma_start(wql_sb[:, 0], wql[0])
            nc.vector.memset(scr[:], 0.5)
            nc.vector.memset(vaug[:, :, :, DK:DK + 1], 1.0)
            nc.scalar.activation(scr2[:], scr[:], EXP, scale=0.125)
            # causal mask generated on-chip: it[p, f] = f - p, then
            # mask[o] = (it >= o*128) -- saves a 0.5MB DMA in the
            # bandwidth-saturated startup window
            nc.gpsimd.iota(it_sb[:], [[1, TQ]], base=0, channel_multiplier=-1)
            for o4 in range(4):
                nc.vector.tensor_scalar(
                    mask_sb[:, o4, :], it_sb[:], float(o4 * TK), None,
                    mybir.AluOpType.is_ge)
            xq0 = stage.tile([128, NKD, TQ], BF16, name="xq_0", tag="xq")
            xk0 = stage.tile([128, NKD, TQ], BF16, name="xk_0", tag="xk")
            xv0 = stage.tile([128, NKD, TQ], BF16, name="xv_0", tag="xv")
            for q in range(4):
                nc.sync.dma_start(xq0[:, 2 * q:2 * q + 2, :],
                                  xq_r[:, 2 * q:2 * q + 2, 0:TQ])
            nc.sync.dma_start(wkl_sb[:, 0], wkl[0])
            for q in range(4):
                nc.sync.dma_start(xk0[:, 2 * q:2 * q + 2, :],
                                  xk_r[:, 2 * q:2 * q + 2, 0:TQ])
            nc.sync.dma_start(wql_sb[:, 1], wql[1])
            nc.sync.dma_start(wkl_sb[:, 1], wkl[1])
            nc.sync.dma_start(wvr_sb[:], wvr.rearrange("(k p) e -> p k e", p=128))
            for hh2 in range(2):
                nc.sync.dma_start(xv0[:, 4 * hh2:4 * hh2 + 4, :],
                                  xv_r[:, 4 * hh2:4 * hh2 + 4, 0:TQ])
            stages[0] = {"q": xq0, "k": xk0, "v": xv0}
            emit_load(1, halves=False)
            nc.sync.dma_start(ident_sb[:], ident)
            nc.sync.dma_start(wor_sb[:], wor.rearrange("(j p) f -> p j f", p=128))

            emit_proj_qk_group(0, "q", wql_sb, None, 0, nc.vector)
            emit_proj_qk_group(0, "k", wkl_sb, kT_sb, 0, nc.vector)
            emit_proj_qk_group(0, "q", wql_sb, None, 1, nc.vector)
            emit_proj_qk_group(0, "k", wkl_sb, kT_sb, 1, nc.vector)

            # ---- main loop ----
            # Scores are zipped with fill work (this chunk's V projection,
            # the previous chunk's deferred transpose/outproj, the next
            # chunk's Q/K projection).  PV groups are emitted at their
            # earliest-ready point: group u right after diagonal score tile
            # 4n+u.  transpose/outproj of chunk n are deferred into chunk
            # n+1's fill (except the last chunk, which trails one step
            # behind its PV groups).
            deferred = []
            pre = {0: [], 1: []}
            from collections import deque
            for n in range(NQ):
                if n + 2 < NQ:
                    emit_load(n + 2, halves=False)
                o_nrm = onrm_pool.tile([128, 4, E], BF16, name=f"onrm_{n}",
                                       tag="on")
                o_sb = osb_pool.tile([128, 4, D], BF16, name=f"osb_{n}",
                                     tag="osb")
                fill = deque(projv_thunks(n))
                if n + 1 < NQ:
                    fill.extend(qk_thunks(n + 1))
                fill.extend(deferred)
                deferred = []
                n_tiles = 4 * n + 4
                p2s = pre
                pre = {0: [], 1: []}
                # pre-score requests for chunk n+1 (first 4 tiles) once this
                # chunk's fill is exhausted (guarantees q/k of n+1 emitted)
                pre_req = deque(range(7)) if n + 1 < NQ else deque()
                i0 = len(p2s[0])
                if n == NQ - 1:
                    # last chunk: all pair-0 scores first, then pair-1 scores
                    # zipped with pair-0 PV, then pair-1 PV with the
                    # transpose/outproj cascade.  Pair-0 exps complete while
                    # the pair-1 scores stream, so PV never outruns ACT.
                    nA = n_tiles - i0
                    for idx, i in enumerate(range(i0, n_tiles)):
                        p2s[0].append(emit_score_tile(n, 0, i))
                        k = -(-len(fill) // (2 * nA - idx))
                        for _ in range(k):
                            if fill:
                                fill.popleft()()
                    for i in range(i0, n_tiles):
                        p2s[1].append(emit_score_tile(n, 1, i))
                        k = -(-len(fill) // (n_tiles - i))
                        for _ in range(k):
                            if fill:
                                fill.popleft()()
                        if i >= 4 * n + 1:
                            u = i - 4 * n - 1
                            emit_pv_group(n, 0, u, p2s[0], o_nrm)
                            emit_pv_group(n, 1, u, p2s[0], o_nrm)
                    emit_pv_group(n, 0, 3, p2s[0], o_nrm)
                    emit_pv_group(n, 1, 3, p2s[0], o_nrm)
                    for u in range(3):
                        emit_pv_group(n, 2, u, p2s[1], o_nrm)
                        emit_pv_group(n, 3, u, p2s[1], o_nrm)
                        oT = emit_tr_u(n, u, o_nrm)
                        emit_op_u(n, u, oT, o_sb)
                        emit_out_dma_u(n, o_sb, u)
                    m = 4 * n + 3
                    _lbl(f"tr{n}u3")
                    oT = otp.tile([128, 2, TK], BF16, name=f"oT_{m}", tag="oT")
                    tp = psum.tile([128, TK], BF16, name=f"tp_{m}_0",
                                   tag="s2", bufs=2)
                    nc.tensor.transpose(tp[:], o_nrm[:, 3, 0:TK], ident_sb[:])
                    nc.scalar.copy(oT[:, 0, :], tp[:])
                    emit_pv_group(n, 2, 3, p2s[1], o_nrm)
                    emit_pv_group(n, 3, 3, p2s[1], o_nrm)
                    _lbl(f"tr{n}u3")
                    tp = psum.tile([128, TK], BF16, name=f"tp_{m}_1",
                                   tag="s2", bufs=2)
                    nc.tensor.transpose(tp[:], o_nrm[:, 3, TK:2 * TK],
                                        ident_sb[:])
                    nc.scalar.copy(oT[:, 1, :], tp[:])
                    _lbl(f"op{n}u3")
                    for c in range(2):
                        op = psum.tile([128, TQ], F32, name=f"op_{m}_{c}",
                                       tag="s", bufs=2)
                        for jj in range(2):
                            nc.tensor.matmul(
                                op[:], oT[:, jj, :],
                                wor_sb[:, jj, c * TQ:(c + 1) * TQ],
                                start=(jj == 0), stop=(jj == 1),
                            )
                        nc.vector.tensor_copy(
                            o_sb[:, 3, c * TQ:(c + 1) * TQ], op[:])
                        nc.sync.dma_start(
                            out[n * TQ + 3 * TK:n * TQ + 4 * TK,
                                c * TQ:(c + 1) * TQ],
                            o_sb[:, 3, c * TQ:(c + 1) * TQ])
                    continue
                next_u = 0
                lag = 1 if n == 0 else 0
                for i in range(i0, n_tiles):
                    p2s[0].append(emit_score_tile(n, 0, i))
                    if i - lag >= 0:
                        p2s[1].append(emit_score_tile(n, 1, i - lag))
                    o = i - 4 * n
                    tiles_left = n_tiles - i
                    k = -(-len(fill) // tiles_left)
                    for _ in range(k):
                        if fill:
                            fill.popleft()()
                    while next_u <= i - 4 * n - 1 - lag:
                        u = next_u
                        for h in range(HPG):
                            emit_pv_group(n, h, u, p2s[h // 2], o_nrm)
                        next_u += 1
                    if o >= 0 and not fill and pre_req:
                        ii = pre_req.popleft()
                        pre[0].append(emit_score_tile(n + 1, 0, ii))
                        pre[1].append(emit_score_tile(n + 1, 1, ii))
                if True:
                    while fill:
                        fill.popleft()()
                    if lag:
                        p2s[1].append(emit_score_tile(n, 1, n_tiles - 1))
                    while pre_req:
                        ii = pre_req.popleft()
                        pre[0].append(emit_score_tile(n + 1, 0, ii))
                        pre[1].append(emit_score_tile(n + 1, 1, ii))
                    while next_u <= 3:
                        for h in range(HPG):
                            emit_pv_group(n, h, next_u, p2s[h // 2], o_nrm)
                        next_u += 1

                    def mk_units(n, o_nrm, o_sb):
                        units = []
                        st = {}
                        for u in range(4):
                            def tr(u=u):
                                st[u] = emit_tr_u(n, u, o_nrm)
                            def op(u=u):
                                emit_op_u(n, u, st[u], o_sb)
                                if u % 2 == 1:
                                    emit_out_dma(n, o_sb, u - 1)
                            units += [tr, op]
                        return units
                    deferred = mk_units(n, o_nrm, o_sb)

    nc.compile()
    return nc


def _get_nc():
    global _NC_CACHE
    if _NC_CACHE is None:
        _NC_CACHE = _build()
    return _NC_CACHE


def kernel(query, key, value, mask, Wq, Wk, Wv, Wo):
    import ml_dtypes
    from concourse.bass_utils import run_bass_kernel_spmd

    BF = ml_dtypes.bfloat16

    query = np.asarray(query, dtype=np.float32)
    key = np.asarray(key, dtype=np.float32)
    value = np.asarray(value, dtype=np.float32)
    mask = np.asarray(mask)
    Wq = np.asarray(Wq, dtype=np.float32)
    Wk = np.asarray(Wk, dtype=np.float32)
    Wv = np.asarray(Wv, dtype=np.float32)
    Wo = np.asarray(Wo, dtype=np.float32)

    ident = np.eye(128, dtype=BF)

    xT = {}
    for b in range(B):
        xT[("q", b)] = np.ascontiguousarray(query[b].T.astype(BF))
        xT[("k", b)] = np.ascontiguousarray(key[b].T.astype(BF))
        xT[("v", b)] = np.ascontiguousarray(value[b].T.astype(BF))

    in_maps = []
    for core in range(N_CORES):
        b, g = divmod(core, G)
        sl = slice(g * E, (g + 1) * E)
        in_maps.append({
            "xqT": xT[("q", b)],
            "xkT": xT[("k", b)],
            "xvT": xT[("v", b)],
            "wql": np.ascontiguousarray(
                Wq[sl, :].T.astype(BF).reshape(NKD, 128, 2, 128)
                .transpose(2, 1, 0, 3)),
            "wkl": np.ascontiguousarray(
                Wk[sl, :].T.astype(BF).reshape(NKD, 128, 2, 128)
                .transpose(2, 1, 0, 3)),
            "wvr": np.ascontiguousarray(Wv[sl, :].T.astype(BF)),
            "wor": np.ascontiguousarray(Wo[:, sl].T.astype(BF)),
            "ident": ident,
        })

    nc = _get_nc()
    res = run_bass_kernel_spmd(nc, in_maps, core_ids=list(range(N_CORES)))

    out = np.zeros((B, S, D), dtype=np.float32)
    for core in range(N_CORES):
        out[core // G] += res.results[core]["out"].astype(np.float32)
    return out
